# revision 2
# baseline (speedup 1.0000x reference)
"""Trainium2 Bass kernel for AdaptConv-style GNN message passing.

Reference computation (per batch element b):
    h   = x @ W.T + b                       # [N, OUT]
    hn  = h / max(||h||_row, 1e-12)         # row-wise L2 normalize
    cos = hn @ hn.T                         # [N, N]
    out = relu((edge_weight * cos) @ h)     # [N, OUT]

Sharding: pure data-parallel over batch B=8 across the 8 NeuronCores
(no collectives).  Host-side layout preprocessing (part of the sharding
strategy): each core receives
    et   = edge_weight[b].T     [N, N]   (so the gated matrix is produced
                                          directly in the [q, p] layout the
                                          aggregation matmul contracts over)
    xt   = x[b].T               [IN, N]
    wt   = W.T                  [IN, OUT]
    bias = b.reshape(OUT, 1)
and returns outT = relu(out).T as [OUT, N]; the host transposes back.

On-chip dataflow per core (all matmuls bf16, fp32 PSUM accumulation):
    hT[o, n]   = wt.T @ xt + bias          (TensorE + ScalarE bias)
    h_rm tiles = PE-transpose(hT)          (row-major h, agg stationary)
    norms      = ScalarE Square+accum, Sqrt; VectorE max/reciprocal
    hnT        = PE-transpose(h_rm * r)
    loop q in 16 row-bands of et (1 MB contiguous DMA each):
      loop pc in 4 chunks of 512:
        cosT[q', p'] = hnT[:, q]^T @ hnT[:, pc]      (PE -> PSUM)
        gT           = et_band[:, pc] * cosT         (DVE -> bf16 SBUF)
        outT[:, pc] += h_rm[q]^T @ gT                (PE, PSUM accum)
    relu epilogue (ScalarE) + DMA out.

The 16 MB/core edge-weight stream is the roofline (~47 us at ~358 GB/s
per-core HBM bandwidth); PE (~30 us) and DVE (~42 us) hide underneath.
"""

import numpy as np

import concourse.bass as bass
import concourse.mybir as mybir
import concourse.tile as tile
from concourse import bacc
from concourse.bass_utils import run_bass_kernel_spmd
from concourse.masks import make_identity

B, N, IN, OUT = 8, 2048, 128, 128
NQ = N // 128  # 16 q-tiles (rows of et band / contraction tiles)
NPC = N // 512  # 4 p-chunks (output column chunks)
FP32 = mybir.dt.float32
BF16 = mybir.dt.bfloat16
AF = mybir.ActivationFunctionType
EPS = 1e-12

CORE_IDS = list(range(8))


def build_nc():
    """Build + compile the single-core Bass graph (same graph runs SPMD on 8 cores)."""
    from contextlib import ExitStack

    nc = bacc.Bacc("TRN2", target_bir_lowering=False, debug=False, num_devices=8)

    et = nc.dram_tensor("et", [N, N], FP32, kind="ExternalInput").ap()
    xt = nc.dram_tensor("xt", [IN, N], FP32, kind="ExternalInput").ap()
    wt = nc.dram_tensor("wt", [IN, OUT], FP32, kind="ExternalInput").ap()
    bias = nc.dram_tensor("bias", [OUT, 1], FP32, kind="ExternalInput").ap()
    out = nc.dram_tensor("out", [OUT, N], FP32, kind="ExternalOutput").ap()

    with tile.TileContext(nc) as tc, ExitStack() as ctx:
        singles = ctx.enter_context(tc.tile_pool(name="singles", bufs=1))
        ident = singles.tile([128, 128], BF16, tag="ident")
        make_identity(nc, ident[:])

        hnT = singles.tile([128, N], BF16, tag="hnT")
        hrm = [
            singles.tile([128, OUT], BF16, tag=f"hrm{i}", name=f"hrm{i}")
            for i in range(NQ)
        ]
        bias_sb = singles.tile([OUT, 1], FP32, tag="bias")
        nc.sync.dma_start(bias_sb[:], bias)

        # ---------------- prologue: h, norms, hn (scoped pools) ----------------
        with ExitStack() as pctx:
            pro = pctx.enter_context(tc.tile_pool(name="pro", bufs=2))
            ppsum = pctx.enter_context(tc.tile_pool(name="ppsum", bufs=2, space="PSUM"))

            xt_f = pro.tile([IN, N], FP32, tag="xt_f")
            nc.sync.dma_start(xt_f[:], xt)
            xt_b = pro.tile([IN, N], BF16, tag="xt_b")
            nc.vector.tensor_copy(xt_b[:], xt_f[:])

            wt_f = pro.tile([IN, OUT], FP32, tag="wt_f")
            nc.sync.dma_start(wt_f[:], wt)
            wt_b = pro.tile([IN, OUT], BF16, tag="wt_b")
            nc.vector.tensor_copy(wt_b[:], wt_f[:])

            hT = pro.tile([128, N], BF16, tag="hT")
            for c in range(N // 512):
                ps = ppsum.tile([OUT, 512], FP32, tag="hT_ps")
                nc.tensor.matmul(
                    ps[:], wt_b[:], xt_b[:, c * 512 : (c + 1) * 512],
                    start=True, stop=True,
                )
                # hT = psum + bias (per-partition bias along OUT)
                nc.scalar.activation(
                    hT[:, c * 512 : (c + 1) * 512], ps[:], AF.Identity,
                    bias=bias_sb[:], scale=1.0,
                )

            for i in range(NQ):
                tp = ppsum.tile([128, 128], BF16, tag="tp")
                nc.tensor.transpose(tp[:], hT[:, i * 128 : (i + 1) * 128], ident[:])
                nc.any.tensor_copy(hrm[i][:], tp[:])

                sq = pro.tile([128, OUT], BF16, tag="sq")
                s = pro.tile([128, 1], FP32, tag="s")
                nc.scalar.activation(sq[:], hrm[i][:], AF.Square, accum_out=s[:])
                nrm = pro.tile([128, 1], FP32, tag="nrm")
                nc.scalar.activation(nrm[:], s[:], AF.Sqrt)
                nrm2 = pro.tile([128, 1], FP32, tag="nrm2")
                nc.vector.tensor_scalar_max(nrm2[:], nrm[:], EPS)
                r = pro.tile([128, 1], FP32, tag="r")
                nc.vector.reciprocal(r[:], nrm2[:])

                hn_i = pro.tile([128, OUT], BF16, tag="hn_i")
                nc.vector.tensor_scalar_mul(hn_i[:], hrm[i][:], r[:])
                tp2 = ppsum.tile([128, 128], BF16, tag="tp")
                nc.tensor.transpose(tp2[:], hn_i[:], ident[:])
                nc.any.tensor_copy(hnT[:, i * 128 : (i + 1) * 128], tp2[:])

        # ---------------- main loop: stream et, gate, aggregate ----------------
        etp = ctx.enter_context(tc.tile_pool(name="etp", bufs=3))
        gtp = ctx.enter_context(tc.tile_pool(name="gtp", bufs=3))
        cps_pool = ctx.enter_context(tc.tile_pool(name="cps", bufs=3, space="PSUM"))
        out_ps = ctx.enter_context(tc.tile_pool(name="outps", bufs=1, space="PSUM"))
        osb = ctx.enter_context(tc.tile_pool(name="osb", bufs=2))

        outT = [
            out_ps.tile([OUT, 512], FP32, tag=f"outT{pc}", name=f"outT{pc}")
            for pc in range(NPC)
        ]

        for q in range(NQ):
            etb = etp.tile([128, N], FP32, tag="etb")
            nc.sync.dma_start(etb[:], et[q * 128 : (q + 1) * 128, :])
            for pc in range(NPC):
                cps = cps_pool.tile([128, 512], FP32, tag="cps")
                nc.tensor.matmul(
                    cps[:],
                    hnT[:, q * 128 : (q + 1) * 128],
                    hnT[:, pc * 512 : (pc + 1) * 512],
                    start=True, stop=True,
                )
                gt = gtp.tile([128, 512], BF16, tag="gt")
                nc.vector.tensor_mul(gt[:], cps[:], etb[:, pc * 512 : (pc + 1) * 512])
                nc.tensor.matmul(
                    outT[pc][:], hrm[q][:], gt[:],
                    start=(q == 0), stop=(q == NQ - 1),
                )

        for pc in range(NPC):
            ob = osb.tile([OUT, 512], FP32, tag="ob")
            nc.scalar.activation(ob[:], outT[pc][:], AF.Relu)
            nc.sync.dma_start(out[:, pc * 512 : (pc + 1) * 512], ob[:])

    nc.compile()
    return nc


_NC_CACHE = None


def _get_nc():
    global _NC_CACHE
    if _NC_CACHE is None:
        _NC_CACHE = build_nc()
    return _NC_CACHE


def make_in_maps(x, edge_weight, W, b):
    x = np.asarray(x, dtype=np.float32)
    edge_weight = np.asarray(edge_weight, dtype=np.float32)
    W = np.asarray(W, dtype=np.float32)
    b = np.asarray(b, dtype=np.float32)
    wt = np.ascontiguousarray(W.T)
    bias = np.ascontiguousarray(b.reshape(OUT, 1))
    in_maps = []
    for core in CORE_IDS:
        in_maps.append(
            {
                "et": np.ascontiguousarray(edge_weight[core].T),
                "xt": np.ascontiguousarray(x[core].T),
                "wt": wt,
                "bias": bias,
            }
        )
    return in_maps


def kernel(x, edge_weight, W, b):
    nc = _get_nc()
    in_maps = make_in_maps(x, edge_weight, W, b)
    res = run_bass_kernel_spmd(nc, in_maps, core_ids=CORE_IDS)
    out = np.stack(
        [np.ascontiguousarray(res.results[i]["out"].T) for i in range(len(CORE_IDS))]
    )
    return out.astype(np.float32, copy=False)


# revision 3
# speedup vs baseline: 1.6676x; 1.6676x over previous
"""Trainium2 Bass kernel for AdaptConv-style GNN message passing.

Reference computation (per batch element b):
    h   = x @ W.T + b                       # [N, OUT]
    hn  = h / max(||h||_row, 1e-12)         # row-wise L2 normalize
    cos = hn @ hn.T                         # [N, N]
    out = relu((edge_weight * cos) @ h)     # [N, OUT]

Sharding: pure data-parallel over batch B=8 across the 8 NeuronCores
(no collectives).  Host-side layout preprocessing (part of the sharding
strategy): each core receives
    et   = edge_weight[b].T     [N, N]   (so the gated matrix is produced
                                          directly in the [q, p] layout the
                                          aggregation matmul contracts over)
    xt   = x[b].T               [IN, N]
    wt   = W.T                  [IN, OUT]
    bias = b.reshape(OUT, 1)
and returns outT = relu(out).T as [OUT, N]; the host transposes back.

On-chip dataflow per core (all matmuls bf16, fp32 PSUM accumulation):
    hT[o, n]   = wt.T @ xt + bias          (TensorE + ScalarE bias)
    h_rm tiles = PE-transpose(hT)          (row-major h, agg stationary)
    norms      = ScalarE Square+accum -> batched Sqrt/max/reciprocal [128,16]
    hnT        = PE-transpose(h_rm * r)
    loop q in 16 row-bands of et (1 MB contiguous DMA each, 8-deep buffer
    so the stream starts at t~0 and overlaps the prologue):
      loop half in 2 chunks of 1024:
        cosT[q', p'] = hnT[:, q]^T @ hnT[:, half]    (2 matmuls -> PSUM)
        gT           = et_band[:, half] * cosT       (1 DVE op -> bf16 SBUF)
        outT[:, pc] += h_rm[q]^T @ gT[:, 512-slice]  (2 matmuls, PSUM accum)
    relu epilogue (ScalarE) + DMA out.

The 16 MB/core edge-weight stream is the roofline (~47 us at ~358 GB/s
per-core HBM bandwidth); PE (~30 us) and DVE (~40 us) hide underneath.
"""

import numpy as np

import concourse.bass as bass
import concourse.mybir as mybir
import concourse.tile as tile
from concourse import bacc
from concourse.bass_utils import run_bass_kernel_spmd
from concourse.masks import make_identity

B, N, IN, OUT = 8, 2048, 128, 128
NQ = N // 128  # 16 q-tiles (rows of et band / contraction tiles)
NPC = N // 512  # 4 p-chunks (output column chunks)
FP32 = mybir.dt.float32
BF16 = mybir.dt.bfloat16
AF = mybir.ActivationFunctionType
EPS = 1e-12

CORE_IDS = list(range(8))


def build_nc():
    """Build + compile the single-core Bass graph (same graph runs SPMD on 8 cores)."""
    from contextlib import ExitStack

    nc = bacc.Bacc("TRN2", target_bir_lowering=False, debug=False, num_devices=8)

    et = nc.dram_tensor("et", [N, N], FP32, kind="ExternalInput").ap()
    xt = nc.dram_tensor("xt", [IN, N], FP32, kind="ExternalInput").ap()
    wt = nc.dram_tensor("wt", [IN, OUT], FP32, kind="ExternalInput").ap()
    bias = nc.dram_tensor("bias", [OUT, 1], FP32, kind="ExternalInput").ap()
    out = nc.dram_tensor("out", [OUT, N], FP32, kind="ExternalOutput").ap()

    with tile.TileContext(nc) as tc, ExitStack() as ctx:
        singles = ctx.enter_context(tc.tile_pool(name="singles", bufs=1))
        # et stream pool FIRST so its SBUF range never aliases prologue
        # scratch (WAR deps would stall the stream behind the prologue).
        etp = ctx.enter_context(tc.tile_pool(name="etp", bufs=8))
        gtp = ctx.enter_context(tc.tile_pool(name="gtp", bufs=3))
        osb = ctx.enter_context(tc.tile_pool(name="osb", bufs=2))

        ident = singles.tile([128, 128], BF16, tag="ident")
        make_identity(nc, ident[:])

        hnT = singles.tile([128, N], BF16, tag="hnT")
        hrm = [
            singles.tile([128, OUT], BF16, tag=f"hrm{i}", name=f"hrm{i}")
            for i in range(NQ)
        ]
        bias_sb = singles.tile([OUT, 1], FP32, tag="bias")
        nc.sync.dma_start(bias_sb[:], bias)
        s_acc = singles.tile([128, NQ], FP32, tag="s_acc")
        s_nrm = singles.tile([128, NQ], FP32, tag="s_nrm")
        s_max = singles.tile([128, NQ], FP32, tag="s_max")
        r_inv = singles.tile([128, NQ], FP32, tag="r_inv")

        # ---------------- prologue: h, norms, hn (scoped pools) ----------------
        with ExitStack() as pctx:
            pro = pctx.enter_context(tc.tile_pool(name="pro", bufs=2))
            ppsum = pctx.enter_context(tc.tile_pool(name="ppsum", bufs=2, space="PSUM"))

            xt_f = pro.tile([IN, N], FP32, tag="xt_f")
            nc.sync.dma_start(xt_f[:], xt)
            xt_b = pro.tile([IN, N], BF16, tag="xt_b")
            nc.vector.tensor_copy(xt_b[:], xt_f[:])

            wt_f = pro.tile([IN, OUT], FP32, tag="wt_f")
            nc.sync.dma_start(wt_f[:], wt)
            wt_b = pro.tile([IN, OUT], BF16, tag="wt_b")
            nc.vector.tensor_copy(wt_b[:], wt_f[:])

            hT = pro.tile([128, N], BF16, tag="hT")
            for c in range(N // 512):
                ps = ppsum.tile([OUT, 512], FP32, tag="hT_ps")
                nc.tensor.matmul(
                    ps[:], wt_b[:], xt_b[:, c * 512 : (c + 1) * 512],
                    start=True, stop=True,
                )
                # hT = psum + bias (per-partition bias along OUT)
                nc.scalar.activation(
                    hT[:, c * 512 : (c + 1) * 512], ps[:], AF.Identity,
                    bias=bias_sb[:], scale=1.0,
                )

            # stage 1: all transposes hT -> row-major h tiles
            for i in range(NQ):
                tp = ppsum.tile([128, 128], BF16, tag="tp", bufs=4)
                nc.tensor.transpose(tp[:], hT[:, i * 128 : (i + 1) * 128], ident[:])
                nc.any.tensor_copy(hrm[i][:], tp[:])

            # stage 2: squared row norms into one [128, 16] tile
            for i in range(NQ):
                sq = pro.tile([128, OUT], BF16, tag="sq", bufs=4)
                nc.scalar.activation(
                    sq[:], hrm[i][:], AF.Square, accum_out=s_acc[:, i : i + 1]
                )
            # stage 3: batched sqrt -> clamp -> reciprocal
            nc.scalar.activation(s_nrm[:], s_acc[:], AF.Sqrt)
            nc.vector.tensor_scalar_max(s_max[:], s_nrm[:], EPS)
            nc.vector.reciprocal(r_inv[:], s_max[:])

            # stage 4: hn tiles + transpose back into hnT
            for i in range(NQ):
                hn_i = pro.tile([128, OUT], BF16, tag="hn_i", bufs=4)
                nc.vector.tensor_scalar_mul(hn_i[:], hrm[i][:], r_inv[:, i : i + 1])
                tp2 = ppsum.tile([128, 128], BF16, tag="tp", bufs=4)
                nc.tensor.transpose(tp2[:], hn_i[:], ident[:])
                nc.any.tensor_copy(hnT[:, i * 128 : (i + 1) * 128], tp2[:])

        # ---------------- main loop: stream et, gate, aggregate ----------------
        # PSUM: 2 x [128,1024] cos (4 banks) + 4 x [128,512] outT (4 banks) = 8
        cps_pool = ctx.enter_context(tc.tile_pool(name="cps", bufs=2, space="PSUM"))
        out_ps = ctx.enter_context(tc.tile_pool(name="outps", bufs=1, space="PSUM"))

        outT = [
            out_ps.tile([OUT, 512], FP32, tag=f"outT{pc}", name=f"outT{pc}")
            for pc in range(NPC)
        ]

        for q in range(NQ):
            etb = etp.tile([128, N], FP32, tag="etb")
            nc.sync.dma_start(etb[:], et[q * 128 : (q + 1) * 128, :])
            for h in range(2):
                cps = cps_pool.tile([128, 1024], FP32, tag="cps")
                for j in range(2):
                    o = h * 1024 + j * 512
                    nc.tensor.matmul(
                        cps[:, j * 512 : (j + 1) * 512],
                        hnT[:, q * 128 : (q + 1) * 128],
                        hnT[:, o : o + 512],
                        start=True, stop=True,
                    )
                gt = gtp.tile([128, 1024], BF16, tag="gt")
                nc.vector.tensor_mul(
                    gt[:], cps[:], etb[:, h * 1024 : (h + 1) * 1024]
                )
                for j in range(2):
                    nc.tensor.matmul(
                        outT[2 * h + j][:],
                        hrm[q][:],
                        gt[:, j * 512 : (j + 1) * 512],
                        start=(q == 0), stop=(q == NQ - 1),
                    )

        for pc in range(NPC):
            ob = osb.tile([OUT, 512], FP32, tag="ob")
            nc.scalar.activation(ob[:], outT[pc][:], AF.Relu)
            nc.sync.dma_start(out[:, pc * 512 : (pc + 1) * 512], ob[:])

    nc.compile()
    return nc


_NC_CACHE = None


def _get_nc():
    global _NC_CACHE
    if _NC_CACHE is None:
        _NC_CACHE = build_nc()
    return _NC_CACHE


def make_in_maps(x, edge_weight, W, b):
    x = np.asarray(x, dtype=np.float32)
    edge_weight = np.asarray(edge_weight, dtype=np.float32)
    W = np.asarray(W, dtype=np.float32)
    b = np.asarray(b, dtype=np.float32)
    wt = np.ascontiguousarray(W.T)
    bias = np.ascontiguousarray(b.reshape(OUT, 1))
    in_maps = []
    for core in CORE_IDS:
        in_maps.append(
            {
                "et": np.ascontiguousarray(edge_weight[core].T),
                "xt": np.ascontiguousarray(x[core].T),
                "wt": wt,
                "bias": bias,
            }
        )
    return in_maps


def kernel(x, edge_weight, W, b):
    nc = _get_nc()
    in_maps = make_in_maps(x, edge_weight, W, b)
    res = run_bass_kernel_spmd(nc, in_maps, core_ids=CORE_IDS)
    out = np.stack(
        [np.ascontiguousarray(res.results[i]["out"].T) for i in range(len(CORE_IDS))]
    )
    return out.astype(np.float32, copy=False)
